# revision 15
# baseline (speedup 1.0000x reference)
"""Trainium2 Bass kernel for BertTempRel-style span-mean + MLP + softmax.

Reference computation (per batch row b of x[B, S, D]):
    e1 = mean(x[b, lo1:hi1, :]),  e2 = mean(x[b, lo2:hi2, :]),  cls = x[b, 0, :]
    (hi = max(hi, lo+1) empty-span guard)
    feat = concat([e1, e2, cls])            # [3D]
    out = softmax(relu(relu(feat@W1+b1)@W2+b2)@W3+b3)

Strategy: pure data-parallel over 8 NeuronCores. Only ~54% of x's rows are
inside the two spans (plus CLS), so instead of streaming all of x, each
core issues one SWDGE dma_gather per PAIR of batch rows that reads exactly
the union-of-spans rows into packed SBUF slots. The host derives the row
lists from the tiny span tensors, sorts the 1024 batch rows by union size
and deals them round-robin across cores (so the 8 rows sharing a loop
position have near-equal counts), then pairs head and tail positions so
every gather moves a near-constant ~560 rows. The per-position counts are
baked into the program as immediates (SPMD-safe: register-valued DMA
operands fail on this deployment); shorter rows pad with dummy row-0
reads. The 0/1 span masks (3 per batch row, 6 per pair) are precomputed
host-side directly in the transposed [slot, mask] layout the TensorEngine
needs, so the device does no mask computation at all: per pair it runs
cq ~= 5 accumulating matmuls (stationary masks x moving gathered rows),
scales the [6, D] PSUM result by host-provided 1/count factors, transposes
batches of 12 rows back to feature-major layout, and finishes with the
128-row MLP + softmax as a handful of matmuls.
"""

import sys

if "/opt/trn_rl_repo" not in sys.path:
    sys.path.insert(0, "/opt/trn_rl_repo")

import numpy as np

from concourse import bacc, bass, mybir, tile
from concourse.bass_utils import run_bass_kernel_spmd
from concourse.masks import make_identity

F32 = mybir.dt.float32
F32R = mybir.dt.float32r
I16 = mybir.dt.int16
I32 = mybir.dt.int32
OP = mybir.AluOpType
AF = mybir.ActivationFunctionType

N_CORES = 8
B_FULL, S, D = 1024, 512, 768
H1, H2, H3 = 256, 64, 4
BPC = B_FULL // N_CORES  # batch rows per core (128)
NP = BPC // 2            # gather pairs per core (64)


def derive_layout(e1_span, e2_span):
    """Host-side layout: row lists, core/pair assignment, masks, scales.

    Returns dict with
      rows[c]    = [BPC] global batch row for each storage slot of core c
      pcnt       = [NP, 2] baked per-sub-row counts (shared by all cores)
      cq         = [NP] slot chunks per pair gather
      gidx[c]    = [128, NP, 2*S//16] int16 wrapped gather index lists
      mt[c]      = [128, cqmax, NP, 6] float32 transposed span masks
      scl[c]     = [6, NP] float32 1/count scales (e1, e2, cls=1) x 2 rows
    """
    e1 = np.asarray(e1_span, dtype=np.int64)
    e2 = np.asarray(e2_span, dtype=np.int64)
    pos = np.arange(S)[None, :]
    lo1, hi1 = e1[:, 0:1], np.maximum(e1[:, 1:2], e1[:, 0:1] + 1)
    lo2, hi2 = e2[:, 0:1], np.maximum(e2[:, 1:2], e2[:, 0:1] + 1)
    m1 = (pos >= lo1) & (pos < hi1)
    m2 = (pos >= lo2) & (pos < hi2)
    m = m1 | m2
    m[:, 0] = True  # CLS row
    n = m.sum(axis=1)

    order = np.argsort(-n, kind="stable")
    perm = order.reshape(BPC, N_CORES)          # position i, core c
    counts = n[perm[:, 0]].astype(np.int64)     # descending => max of group

    # head-tail pairing: pair p serves positions (p, BPC-1-p)
    pcnt = np.stack([counts[:NP], counts[BPC - 1 - np.arange(NP)]], axis=1)
    cq = (pcnt.sum(axis=1) + 127) // 128
    cqmax = int(cq.max())

    rows, gidx, mt, scl = [], [], [], []
    for c in range(N_CORES):
        rows_c = np.empty(BPC, dtype=np.int64)
        rows_c[0::2] = perm[:NP, c]
        rows_c[1::2] = perm[BPC - 1 - np.arange(NP), c]
        rows.append(rows_c)

        idx_c = np.zeros((NP, 2 * S), dtype=np.int16)
        mt_c = np.zeros((128, cqmax, NP, 6), dtype=np.float32)
        scl_c = np.empty((6, NP), dtype=np.float32)
        for p in range(NP):
            off = 0
            for jj in range(2):
                gb = rows_c[2 * p + jj]
                u = np.flatnonzero(m[gb])
                k = int(pcnt[p, jj])
                # slot s of this pair holds source row idx within the
                # pair's 2S-row window: sub-row jj lives at jj*S + row
                idx_c[p, off:off + len(u)] = jj * S + u
                # dummy padding reads row 0 of sub-row jj (mask 0)
                idx_c[p, off + len(u):off + k] = jj * S
                slots = off + np.arange(len(u))
                mt_c[slots % 128, slots // 128, p, 3 * jj + 0] = m1[gb, u]
                mt_c[slots % 128, slots // 128, p, 3 * jj + 1] = m2[gb, u]
                mt_c[slots % 128, slots // 128, p, 3 * jj + 2] = (u == 0)
                scl_c[3 * jj + 0, p] = 1.0 / (hi1[gb, 0] - lo1[gb, 0])
                scl_c[3 * jj + 1, p] = 1.0 / (hi2[gb, 0] - lo2[gb, 0])
                scl_c[3 * jj + 2, p] = 1.0
                off += k
        # wrap idx for the Q7 cores: w[p_part, p, cblk] = idx[p, cblk*16 +
        # p_part%16], replicated over the 8 groups of 16 partitions
        w = idx_c.reshape(NP, 2 * S // 16, 16).transpose(2, 0, 1)
        gidx.append(np.ascontiguousarray(np.tile(w, (8, 1, 1))))
        mt.append(mt_c)
        scl.append(scl_c)
    return {"rows": rows, "pcnt": pcnt, "cq": cq, "cqmax": cqmax,
            "gidx": gidx, "mt": mt, "scl": scl}


def build_program(bpc=BPC, s=S, d=D, h1=H1, h2=H2, h3=H3, rep=1,
                  xbufs=3, pcnt=None, cqmax=None):
    """Emit the per-core Bass/Tile program. All 8 cores run it SPMD."""
    if pcnt is None:
        pcnt = _LAYOUT_CACHE["layout"]["pcnt"]
        cqmax = _LAYOUT_CACHE["layout"]["cqmax"]
    np_ = bpc // 2
    dh = d // 2            # moving free-dim per span matmul
    nd = d // 128          # d-chunks of 128
    nf = 3 * d // 128      # feature chunks of 128
    nh1 = h1 // 128
    cq = [(int(pcnt[p, 0] + pcnt[p, 1]) + 127) // 128 for p in range(np_)]
    assert max(cq) <= cqmax

    nc = bacc.Bacc("TRN2", target_bir_lowering=False, debug=False,
                   num_devices=N_CORES, num_swdge_queues=4)

    x_d = nc.dram_tensor("x", [bpc * s, d], F32R, kind="ExternalInput")
    gi_d = nc.dram_tensor("gidx", [128, np_, 2 * s // 16], I16,
                          kind="ExternalInput")
    mt_d = nc.dram_tensor("mt", [128, cqmax, np_, 6], F32R,
                          kind="ExternalInput")
    sc_d = nc.dram_tensor("scl", [6, np_], F32, kind="ExternalInput")
    w1_d = nc.dram_tensor("W1", [nf, 128, h1], F32, kind="ExternalInput")
    b1_d = nc.dram_tensor("b1", [1, h1], F32, kind="ExternalInput")
    w2_d = nc.dram_tensor("W2", [nh1, 128, h2], F32, kind="ExternalInput")
    b2_d = nc.dram_tensor("b2", [1, h2], F32, kind="ExternalInput")
    w3_d = nc.dram_tensor("W3", [h2, h3], F32, kind="ExternalInput")
    b3_d = nc.dram_tensor("b3", [1, h3], F32, kind="ExternalInput")
    out_d = nc.dram_tensor("out", [bpc, h3], F32, kind="ExternalOutput")

    with tile.TileContext(nc) as tc:
        with tc.tile_pool(name="const", bufs=1) as const:
            ident = const.tile([128, 128], F32)
            make_identity(nc, ident[:])

            w1 = const.tile([128, nf, h1], F32)
            nc.sync.dma_start(w1[:], w1_d.ap().rearrange("p k h -> k p h"))
            w2 = const.tile([128, nh1, h2], F32)
            nc.sync.dma_start(w2[:], w2_d.ap().rearrange("p k h -> k p h"))
            w3 = const.tile([h2, h3], F32)
            nc.sync.dma_start(w3[:], w3_d.ap()[:])
            b1r = const.tile([1, h1], F32)
            nc.sync.dma_start(b1r[:], b1_d.ap()[:])
            b2r = const.tile([1, h2], F32)
            nc.sync.dma_start(b2r[:], b2_d.ap()[:])
            b3r = const.tile([1, h3], F32)
            nc.sync.dma_start(b3r[:], b3_d.ap()[:])
            ones = const.tile([1, 128], F32)
            nc.vector.memset(ones[:], 1.0)

            for _rep in range(rep):
                gidx = const.tile([128, np_, 2 * s // 16], I16)
                nc.scalar.dma_start(gidx[:], gi_d.ap()[:])
                mt = const.tile([128, cqmax, np_, 6], F32R)
                nc.sync.dma_start(mt[:], mt_d.ap()[:])
                scl = const.tile([6, np_], F32)
                nc.sync.dma_start(scl[:], sc_d.ap()[:])

                # packT[d_p, dc, b, m]: transposed scaled span sums / cls.
                packT = const.tile([128, nd, bpc, 3], F32)

                # ---- main loop: gather span rows, accumulate sums on PE ----
                with tc.tile_pool(name="xp", bufs=xbufs) as xp, \
                     tc.tile_pool(name="stg", bufs=4) as stg, \
                     tc.tile_pool(name="sps0", bufs=2, space="PSUM") as sps0, \
                     tc.tile_pool(name="sps1", bufs=2, space="PSUM") as sps1, \
                     tc.tile_pool(name="ptp", bufs=2, space="PSUM") as ptp:
                    for p in range(np_):
                        xb = xp.tile([128, cqmax, d], F32R, tag="xb")
                        if _rep == 0 and p < xbufs:
                            # first rotation: ensure padding slots hold
                            # finite data (0 x garbage = NaN on the PE);
                            # memset can't encode f32r, so set as f32 bits
                            nc.vector.memset(xb[:].bitcast(F32), 0.0)
                        nc.gpsimd.dma_gather(
                            xb[:, 0:cq[p], :], x_d.ap()[bass.ts(p, 2 * s)],
                            gidx[:, p, :], 128 * cq[p],
                            int(pcnt[p, 0] + pcnt[p, 1]), d,
                            queue_num=p % 4)

                        sg = stg.tile([6, d], F32, tag="sg")
                        ps0 = sps0.tile([6, dh], F32, tag="ps0")
                        ps1 = sps1.tile([6, dh], F32, tag="ps1")
                        for c in range(cq[p]):
                            lhsT = mt[:, c, p, :]
                            nc.tensor.matmul(ps0[:], lhsT, xb[:, c, 0:dh],
                                             start=(c == 0),
                                             stop=(c == cq[p] - 1))
                            nc.tensor.matmul(ps1[:], lhsT, xb[:, c, dh:d],
                                             start=(c == 0),
                                             stop=(c == cq[p] - 1))
                        # evacuate + scale by 1/cnt (split DVE/ACT)
                        nc.vector.tensor_scalar(sg[:, 0:dh], ps0[:],
                                                scl[:, p:p + 1], None, OP.mult)
                        nc.scalar.mul(sg[:, dh:d], ps1[:], scl[:, p:p + 1])
                        # transpose [6, d] -> nd x [128, 6] into packT
                        for dc in range(nd):
                            pt = ptp.tile([128, 6], F32, tag="pt")
                            nc.tensor.transpose(pt[:], sg[:, bass.ts(dc, 128)],
                                                ident[0:6, 0:6])
                            for jj in range(2):
                                if (dc + jj) % 2 == 0:
                                    nc.vector.tensor_copy(
                                        packT[:, dc, 2 * p + jj, :],
                                        pt[:, 3 * jj:3 * jj + 3])
                                else:
                                    nc.scalar.copy(
                                        packT[:, dc, 2 * p + jj, :],
                                        pt[:, 3 * jj:3 * jj + 3])

                # ---- de-interleave features: featT[f_p, k, b] ----
                featT = const.tile([128, nf, bpc], F32)
                for m in range(3):
                    for dc in range(nd):
                        nc.vector.tensor_copy(featT[:, m * nd + dc, :],
                                              packT[:, dc, :, m])

                # ---- MLP + softmax over all bpc rows at once ----
                h1s = const.tile([bpc, h1], F32)
                h1T = const.tile([128, nh1, bpc], F32)
                h2s = const.tile([bpc, h2], F32)
                h2T = const.tile([h2, bpc], F32)
                probs = const.tile([bpc, h3], F32)
                mx = const.tile([bpc, 1], F32)
                ex = const.tile([bpc, h3], F32)
                sm = const.tile([bpc, 1], F32)
                rc = const.tile([bpc, 1], F32)

                with tc.tile_pool(name="mlpp", bufs=1, space="PSUM") as mp:
                    h1p = mp.tile([bpc, h1], F32, tag="h1p")
                    for k in range(nf):
                        nc.tensor.matmul(h1p[:], featT[:, k, :], w1[:, k, :],
                                         start=(k == 0), stop=False)
                    nc.tensor.matmul(h1p[:], ones[0:1, 0:bpc], b1r[:],
                                     start=False, stop=True)
                    nc.scalar.activation(h1s[:], h1p[:], AF.Relu)

                    for k in range(nh1):
                        tp1 = mp.tile([128, bpc], F32, tag="tp1")
                        nc.tensor.transpose(tp1[:], h1s[:, bass.ts(k, 128)],
                                            ident[0:bpc, 0:bpc])
                        nc.vector.tensor_copy(h1T[:, k, :], tp1[:])

                    h2p = mp.tile([bpc, h2], F32, tag="h2p")
                    for k in range(nh1):
                        nc.tensor.matmul(h2p[:], h1T[:, k, :], w2[:, k, :],
                                         start=(k == 0), stop=False)
                    nc.tensor.matmul(h2p[:], ones[0:1, 0:bpc], b2r[:],
                                     start=False, stop=True)
                    nc.scalar.activation(h2s[:], h2p[:], AF.Relu)

                    tp2 = mp.tile([h2, bpc], F32, tag="tp2")
                    nc.tensor.transpose(tp2[:], h2s[:], ident[0:bpc, 0:bpc])
                    nc.vector.tensor_copy(h2T[:], tp2[:])

                    h3p = mp.tile([bpc, h3], F32, tag="h3p")
                    nc.tensor.matmul(h3p[:], h2T[:], w3[:], start=True,
                                     stop=False)
                    nc.tensor.matmul(h3p[:], ones[0:1, 0:bpc], b3r[:],
                                     start=False, stop=True)

                    # softmax along the 4 logits
                    nc.vector.tensor_reduce(mx[:], h3p[:], mybir.AxisListType.X,
                                            OP.max, negate=True)
                    nc.scalar.activation(ex[:], h3p[:], AF.Exp, bias=mx[:],
                                         scale=1.0)
                    nc.vector.tensor_reduce(sm[:], ex[:], mybir.AxisListType.X,
                                            OP.add)
                    nc.vector.reciprocal(rc[:], sm[:])
                    nc.vector.tensor_scalar(probs[:], ex[:], rc[:], None,
                                            OP.mult)

                nc.sync.dma_start(out_d.ap()[:], probs[:])

    nc.compile()
    return nc


_LAYOUT_CACHE = {}
_NC_CACHE = {}


def _get_program(layout):
    key = tuple(int(c) for c in layout["pcnt"].ravel())
    if _NC_CACHE.get("key") != key:
        _NC_CACHE["nc"] = build_program(pcnt=layout["pcnt"],
                                        cqmax=layout["cqmax"])
        _NC_CACHE["key"] = key
    return _NC_CACHE["nc"]


def make_in_maps(inputs):
    x = np.ascontiguousarray(np.asarray(inputs["x"], dtype=np.float32))
    e1 = np.ascontiguousarray(np.asarray(inputs["e1_span"], dtype=np.int32))
    e2 = np.ascontiguousarray(np.asarray(inputs["e2_span"], dtype=np.int32))
    w1 = np.ascontiguousarray(
        np.asarray(inputs["W1"], dtype=np.float32).reshape(3 * D // 128, 128, H1))
    b1 = np.asarray(inputs["b1"], dtype=np.float32).reshape(1, H1)
    w2 = np.ascontiguousarray(
        np.asarray(inputs["W2"], dtype=np.float32).reshape(H1 // 128, 128, H2))
    b2 = np.asarray(inputs["b2"], dtype=np.float32).reshape(1, H2)
    w3 = np.ascontiguousarray(np.asarray(inputs["W3"], dtype=np.float32))
    b3 = np.asarray(inputs["b3"], dtype=np.float32).reshape(1, H3)

    layout = derive_layout(e1, e2)
    _LAYOUT_CACHE["layout"] = layout

    in_maps = []
    for c in range(N_CORES):
        rows = layout["rows"][c]
        in_maps.append({
            "x": np.ascontiguousarray(x[rows].reshape(BPC * S, D)),
            "gidx": layout["gidx"][c],
            "mt": layout["mt"][c],
            "scl": layout["scl"][c],
            "W1": w1, "b1": b1, "W2": w2, "b2": b2, "W3": w3, "b3": b3,
        })
    return in_maps


def kernel(**inputs) -> np.ndarray:
    in_maps = make_in_maps(inputs)
    layout = _LAYOUT_CACHE["layout"]
    nc = _get_program(layout)
    res = run_bass_kernel_spmd(nc, in_maps, core_ids=list(range(N_CORES)))
    out = np.empty((B_FULL, H3), dtype=np.float32)
    for c in range(N_CORES):
        out[layout["rows"][c]] = res.results[c]["out"]
    return out
